# revision 18
# baseline (speedup 1.0000x reference)
"""GraphSAGE (3-layer, mean aggregation) on 8 Trainium2 NeuronCores.

Strategy (1D graph partitioning, nodes sharded by row across 8 cores):
  - Core c owns nodes [c*12500, (c+1)*12500); edges partitioned by dst.
  - Per layer l:  Y = h @ Wl (dense, PE) -> bf16 y_loc, AllGathered in 4
    range-aligned chunks (chunk k = source windows [25,25,24,24]) so the
    collective overlaps the previous phases; aggregation for range r only
    waits on chunk r.
  - Aggregation: edges grouped by (dst super-window of 8, src range);
    gathered source rows (SWDGE dma_gather, int16 offsets within range)
    are segment-summed by dst via one-hot selection matmuls accumulated
    in packed PSUM tiles ([128, 8, 128] fp32, one slice per window).
  - The one-hot selection planes S depend only on edge_index, so they are
    precomputed HOST-side (identical for all 3 layers) and streamed from
    DRAM as bf16 — no DVE work on the critical path.
  - h_new = relu(seg_sum * inv_deg + h @ Wr + b); layers 1/2 dense is
    window-fused into the previous layer's epilogue.

Edges are packed edge-granularly inside each (sw, range) run using
SPMD-uniform per-(window, range) segment sizes (max over cores), so only
run totals are padded to whole 128-slot blocks. A block may hold edges of
several windows; each window multiplies its own one-hot plane (zeros for
foreign slots).
"""

import os

import numpy as np
import ml_dtypes

P = 128
NCORES = 8
N_NODES = 100000
NLOC = N_NODES // NCORES            # 12500 nodes per core
NW = (NLOC + P - 1) // P            # 98 dst windows per core
NLOCP = NW * P                      # 12544 (padded local nodes)
NRANGE = 4
CHUNK_W = [25, 25, 24, 24]          # source windows per AllGather chunk
CHUNK_START = [0, 25, 50, 74]
CHUNK_ROWS = [w * P for w in CHUNK_W]          # rows per core per chunk
RANGE_ROWS = [NCORES * r for r in CHUNK_ROWS]  # rows per gathered range
NSW = 8                             # dst windows per super-window
DIMS = [(128, 128), (128, 128), (128, 64)]
GCH = 128                           # gathered feature columns (zero-padded)
OUT_CH = 64

LAST_EXEC_TIME_NS = None
LAST_RESULTS = None


def _sw_groups(nw, nsw):
    return [list(range(a, min(a + nsw, nw))) for a in range(0, nw, nsw)]


def _build_program(layout, nw=NW, nlocp=NLOCP, ncores=NCORES,
                   dims=DIMS, debug=False, ablate=()):
    """layout: dict from _preprocess. ablate: subset of
    {"gather", "agg", "dense", "collective"} — skip that phase (wrong
    results; timing attribution only)."""
    import concourse.bacc as bacc
    import concourse.bass as bass
    import concourse.mybir as mybir
    import concourse.tile as tile
    from concourse.masks import make_identity

    dt = mybir.dt
    AF = mybir.ActivationFunctionType
    OP = mybir.AluOpType
    out_ch = dims[-1][1]

    runs = layout["runs"]             # [si][r] = (col0, nblk)
    spans = layout["spans"]           # [w][r] = (b0, b1) global block cols
    plane_base = layout["plane_base"]  # [w][r] = first plane index
    sw_groups = layout["sw_groups"]
    sw_col_start = layout["sw_col_start"]
    total_cols = layout["total_cols"]
    nplanes = layout["nplanes"]
    max_sw_cols = layout["max_sw_cols"]
    smax = layout["smax"]             # max planes per (sw, r)
    gmax = layout["gmax"]             # max blocks per (sw, r)
    nsw = len(sw_groups[0])
    chunk_of_w = []
    for k, cw in enumerate(CHUNK_W):
        chunk_of_w += [k] * cw
    chunk_end = [CHUNK_START[k] + CHUNK_W[k] for k in range(NRANGE)]

    nc = bacc.Bacc("TRN2", target_bir_lowering=False, debug=False,
                   num_devices=ncores, num_swdge_queues=4)

    xT_in = nc.dram_tensor("xT", [P, nlocp], dt.bfloat16,
                           kind="ExternalInput")
    wcat_in = [nc.dram_tensor(f"wcat{l}", [dims[l][0], 2 * dims[l][1]],
                              dt.bfloat16, kind="ExternalInput")
               for l in range(3)]
    bbc_in = [nc.dram_tensor(f"bbc{l}", [P, dims[l][1]], dt.float32,
                             kind="ExternalInput") for l in range(3)]
    idx16_in = nc.dram_tensor("idx16", [P, total_cols * 8], dt.int16,
                              kind="ExternalInput")
    s_in = nc.dram_tensor("splanes", [P, nplanes, P], dt.float8e4,
                          kind="ExternalInput")
    invd_in = nc.dram_tensor("invd", [P, nw], dt.float32,
                             kind="ExternalInput")
    h_out = nc.dram_tensor("h_out", [nlocp, out_ch], dt.float32,
                           kind="ExternalOutput")
    dbg = {}
    if debug:
        for l in range(3):
            for k in range(NRANGE):
                dbg[f"yf{l}_{k}"] = nc.dram_tensor(
                    f"yf{l}_{k}", [RANGE_ROWS[k], GCH], dt.bfloat16,
                    kind="ExternalOutput")

    with tile.TileContext(nc) as tc:
        with (
            tc.tile_pool(name="const", bufs=1) as cpool,
            tc.tile_pool(name="dram", bufs=1, space="DRAM") as dpool,
            tc.tile_pool(name="htr", bufs=4) as htpool,
            tc.tile_pool(name="yt", bufs=4) as ypool,
            tc.tile_pool(name="gat", bufs=5) as gpool,
            tc.tile_pool(name="sel", bufs=3) as spool,
            tc.tile_pool(name="epi", bufs=8) as epool,
            tc.tile_pool(name="pst", bufs=2, space="PSUM") as pt_pool,
            tc.tile_pool(name="psm", bufs=2, space="PSUM") as pmm_pool,
            tc.tile_pool(name="psa", bufs=2, space="PSUM") as pa_pool,
        ):
            ident = cpool.tile([P, P], dt.bfloat16)
            make_identity(nc, ident[:])
            invd_sb = cpool.tile([P, nw], dt.float32)
            nc.sync.dma_start(invd_sb[:], invd_in[:, :])
            xT_sb = cpool.tile([P, nlocp], dt.bfloat16)
            nc.sync.dma_start(xT_sb[:], xT_in[:, :])
            idx_sb = cpool.tile([P, total_cols * 8], dt.int16)
            nc.sync.dma_start(idx_sb[:], idx16_in[:, :])
            wc_sb = []
            bb_sb = []
            for l in range(3):
                w_t = cpool.tile([dims[l][0], 2 * dims[l][1]], dt.bfloat16,
                                 name=f"wc{l}")
                nc.sync.dma_start(w_t[:], wcat_in[l][:, :])
                wc_sb.append(w_t)
                b_t = cpool.tile([P, dims[l][1]], dt.float32, name=f"bb{l}")
                nc.sync.dma_start(b_t[:], bbc_in[l][:, :])
                bb_sb.append(b_t)
            r_res = cpool.tile([P, nw, dims[0][1]], dt.bfloat16)
            if "dense" in ablate:
                nc.vector.memset(r_res[:], 0.0)
            # h for layers 1/2 lives in SBUF (bf16), window-sliced
            h_sb = [cpool.tile([P, nw, dims[0][1]], dt.bfloat16,
                               name=f"hsb{i}") for i in range(2)]
            if "agg" in ablate:
                for t in h_sb:
                    nc.vector.memset(t[:], 0.0)

            y_locs = [[dpool.tile([CHUNK_ROWS[k], GCH], dt.bfloat16,
                                  name=f"y_loc{l}_{k}") for k in range(NRANGE)]
                      for l in range(3)]
            y_fulls = [[dpool.tile([RANGE_ROWS[k], GCH], dt.bfloat16,
                                   addr_space="Shared", name=f"y_full{l}_{k}")
                        for k in range(NRANGE)] for l in range(3)]

            def allgather(l, k):
                if "collective" in ablate:
                    nc.sync.dma_start(
                        y_fulls[l][k][0:CHUNK_ROWS[k], :],
                        y_locs[l][k][:, :])
                else:
                    nc.gpsimd.collective_compute(
                        "AllGather", mybir.AluOpType.bypass,
                        replica_groups=[list(range(ncores))],
                        ins=[y_locs[l][k].opt()],
                        outs=[y_fulls[l][k].opt()])
                if debug:
                    nc.sync.dma_start(dbg[f"yf{l}_{k}"][:, :],
                                      y_fulls[l][k][:, :])

            def dense_w(l, i, lhsT_ap):
                # Y_w = h_w @ Wl (-> bf16 y_loc), r_res_w = h_w @ Wr + b
                # lhsT_ap: [din, 128] stationary (h_w transposed)
                din, dout = dims[l]
                mm = pmm_pool.tile([P, 2 * dout], dt.float32, tag="mm")
                nc.tensor.matmul(mm[:], lhsT=lhsT_ap,
                                 rhs=wc_sb[l][:, :], start=True, stop=True)
                y_t = ypool.tile([P, GCH], dt.bfloat16, tag="y_t")
                nc.scalar.activation(y_t[:, :dout], mm[:, :dout], AF.Copy)
                if dout < GCH:
                    nc.vector.memset(y_t[:, dout:], 0.0)
                k = chunk_of_w[i]
                r0 = (i - CHUNK_START[k]) * P
                nc.sync.dma_start(y_locs[l][k][r0:r0 + P, :], y_t[:])
                nc.vector.tensor_tensor(out=r_res[:, i, :dout],
                                        in0=mm[:, dout:2 * dout],
                                        in1=bb_sb[l][:, :], op=OP.add)

            def dense_w_tr(l, i, h_ap):
                # transpose h_w then dense_w (layers 1/2)
                din = dims[l][0]
                t_ps = pt_pool.tile([P, P], dt.bfloat16, tag="t_ps")
                nc.tensor.transpose(t_ps[:din, :], h_ap, ident[:])
                hT = htpool.tile([P, P], dt.bfloat16, tag="hT")
                nc.vector.tensor_copy(hT[:din, :], t_ps[:din, :])
                dense_w(l, i, hT[:din, :])

            # ---- layer 0 dense from xT (no transpose needed) ----
            # Windows are processed HIGH->LOW everywhere (dense, agg,
            # epilogues) and ranges 3->0, so the AllGather chunk that a
            # layer finishes LAST (chunk 0) is the one the next layer
            # needs LAST — collectives stay off the critical path.
            if "dense" not in ablate:
                for i in range(nw):
                    dense_w(0, i, xT_sb[:, i * P:(i + 1) * P])
                    if i + 1 in chunk_end:
                        allgather(0, chunk_end.index(i + 1))
            elif "collective" not in ablate:
                for k in range(NRANGE):
                    allgather(0, k)

            for l in range(3):
                din, dout = dims[l]

                # ---- aggregation ----
                for si in range(len(sw_groups)):
                    grp = sw_groups[si]
                    c0 = sw_col_start[si]
                    agg = pa_pool.tile([P, nsw, GCH], dt.float32, tag="agg")
                    done = [0] * len(grp)
                    nbw = [sum(spans[w][r][1] - spans[w][r][0]
                               for r in range(NRANGE)) for w in grp]
                    bank_started = [False, False]
                    for r in range(NRANGE):
                        rc0, rblk = runs[si][r]
                        npl = sum(spans[w][r][1] - spans[w][r][0]
                                  for w in grp)
                        if rblk == 0:
                            continue
                        pb0 = plane_base[grp[0]][r]
                        s_t = spool.tile([P, smax, P], dt.float8e4,
                                         tag="s_t")
                        if npl > 0 and "agg" not in ablate:
                            nc.scalar.dma_start(
                                s_t[:, :npl, :],
                                s_in[:, pb0:pb0 + npl, :])
                        g_t = gpool.tile([P, gmax, GCH], dt.bfloat16,
                                         tag="g_t")
                        if "gather" in ablate:
                            nc.vector.memset(g_t[:, :rblk, :], 0.0)
                        else:
                            # split each run across 2 SWDGE queues (desc
                            # gen runs on 2 Q7 cores; 2 parts halves the
                            # per-call fixed cost vs 4); rotate queue
                            # pairs across ranges to balance all 4 rings
                            if rblk <= 1:
                                parts = [(0, rblk)]
                            else:
                                qs = [rblk * i // 2 for i in range(3)]
                                parts = list(zip(qs[:-1], qs[1:]))
                            for q, (b0, b1) in enumerate(parts):
                                nb = b1 - b0
                                nc.gpsimd.dma_gather(
                                    out_ap=g_t[:, b0:b1, :],
                                    in_ap=y_fulls[l][r][:, :],
                                    idxs_ap=idx_sb[:, (rc0 + b0) * 8:
                                                   (rc0 + b1) * 8],
                                    num_idxs=nb * P, num_idxs_reg=nb * P,
                                    elem_size=GCH, single_packet=False,
                                    queue_num=(2 * (r % 2) + q) % 4)
                        if "agg" in ablate:
                            continue
                        for w in grp:
                            wi = w - grp[0]
                            b0, b1 = spans[w][r]
                            if b1 == b0:
                                continue
                            pw = plane_base[w][r] - pb0
                            for k in range(b1 - b0):
                                # start=True clears has_written for the
                                # WHOLE bank, so only the bank's first
                                # matmul in this sw sets it; other
                                # windows' first writes land on cleared
                                # bits = overwrite-then-accumulate.
                                st = not bank_started[wi // 4]
                                bank_started[wi // 4] = True
                                nc.tensor.matmul(
                                    agg[:, wi, :],
                                    lhsT=s_t[:, pw + k, :],
                                    rhs=g_t[:, b0 - rc0 + k, :],
                                    start=st,
                                    stop=(done[wi] == nbw[wi] - 1),
                                    skip_group_check=True)
                                done[wi] += 1
                    if "agg" in ablate:
                        continue
                    # ---- epilogue (+ fused dense of next layer) ----
                    for w in grp:
                        wi = w - grp[0]
                        if l < 2:
                            t_t = epool.tile([P, dout], dt.bfloat16,
                                             tag="t_t")
                            nc.scalar.activation(t_t[:], agg[:, wi, :dout],
                                                 AF.Copy,
                                                 scale=invd_sb[:, w:w + 1])
                            o_t = epool.tile([P, dout], dt.bfloat16,
                                             tag="o_t")
                            nc.vector.tensor_tensor(out=o_t[:], in0=t_t[:],
                                                    in1=r_res[:, w, :dout],
                                                    op=OP.add)
                            nc.vector.tensor_scalar_max(
                                h_sb[l % 2][:, w, :dout], o_t[:], 0.0)
                            if "dense" not in ablate:
                                dense_w_tr(l + 1, w,
                                           h_sb[l % 2][:, w, :dout])
                                if w + 1 in chunk_end:
                                    allgather(l + 1,
                                              chunk_end.index(w + 1))
                        else:
                            t_t = epool.tile([P, dout], dt.float32,
                                             tag="t_t")
                            nc.scalar.activation(t_t[:], agg[:, wi, :dout],
                                                 AF.Copy,
                                                 scale=invd_sb[:, w:w + 1])
                            o_t = epool.tile([P, dout], dt.float32,
                                             tag="o_t")
                            nc.vector.tensor_tensor(out=o_t[:], in0=t_t[:],
                                                    in1=r_res[:, w, :dout],
                                                    op=OP.add)
                            nc.sync.dma_start(h_out[w * P:(w + 1) * P, :],
                                              o_t[:])
                if l < 2 and "dense" in ablate and "collective" not in ablate:
                    for k in range(NRANGE):
                        allgather(l + 1, k)

    nc.compile()
    return nc


def _preprocess(x, src, dst, ncores=NCORES, nloc=NLOC, nw=NW, nlocp=NLOCP,
                nsw=NSW):
    """Pack per-core edge/index/one-hot arrays grouped by
    (super-window, src range/chunk).

    Returns (per_core input dicts, layout dict for _build_program).
    """
    bf16 = ml_dtypes.bfloat16
    chunk_of_w = np.zeros(nw, np.int64)
    for k in range(NRANGE):
        chunk_of_w[CHUNK_START[k]:CHUNK_START[k] + CHUNK_W[k]] = k
    chunk_start_arr = np.asarray(CHUNK_START, np.int64)
    chunk_rows_arr = np.asarray(CHUNK_ROWS, np.int64)

    order = np.argsort(dst, kind="stable")
    src_s = src[order].astype(np.int64)
    dst_s = dst[order].astype(np.int64)
    bounds = np.searchsorted(dst_s, np.arange(ncores + 1) * nloc)

    cores = []
    cnts = np.zeros((ncores, nw, NRANGE), np.int64)
    for c in range(ncores):
        lo, hi = bounds[c], bounds[c + 1]
        s = src_s[lo:hi]
        lcl = dst_s[lo:hi] - c * nloc
        w = lcl // P
        sowner = s // nloc
        s_loc = s - sowner * nloc
        s_w = s_loc // P
        rix = chunk_of_w[s_w]
        # offset of the source row within its gathered range (int16-safe)
        off = (sowner * chunk_rows_arr[rix]
               + (s_w - chunk_start_arr[rix]) * P + s_loc % P)
        key = w * NRANGE + rix
        o2 = np.lexsort((off, key))
        off, lcl, w, rix, key = off[o2], lcl[o2], w[o2], rix[o2], key[o2]
        cnts[c] = np.bincount(key, minlength=nw * NRANGE)\
            .reshape(nw, NRANGE)
        cores.append((off, lcl, w, rix, key))

    # SPMD-uniform segment sizes: max edge count over cores per (w, r)
    seg = cnts.max(axis=0).astype(np.int64)        # [nw, NRANGE]
    seg[:, 0] = np.maximum(seg[:, 0], 1)           # keep every window alive

    sw_groups = _sw_groups(nw, nsw)
    seg_start = np.zeros((nw, NRANGE), np.int64)   # slot offset within run
    run_col = np.zeros((len(sw_groups), NRANGE), np.int64)
    runs = []
    spans = [[None] * NRANGE for _ in range(nw)]   # (b0, b1) global cols
    plane_base = [[0] * (NRANGE) for _ in range(nw)]
    sw_col_start = [0]
    gc = 0
    pc = 0
    smax = 0
    gmax = 0
    for si, grp in enumerate(sw_groups):
        sw_runs = []
        for r in range(NRANGE):
            tot = 0
            for w in grp:
                seg_start[w, r] = tot
                tot += seg[w, r]
            nb = (tot + P - 1) // P
            run_col[si, r] = gc
            npl_run = 0
            for w in grp:
                s0, s1 = seg_start[w, r], seg_start[w, r] + seg[w, r]
                if s1 > s0:
                    b0 = int(gc + s0 // P)
                    b1 = int(gc + (s1 + P - 1) // P)
                else:
                    b0 = b1 = int(gc)
                spans[w][r] = (b0, b1)
                plane_base[w][r] = pc
                pc += b1 - b0
                npl_run += b1 - b0
            smax = max(smax, npl_run)
            gmax = max(gmax, int(nb))
            sw_runs.append((int(gc), int(nb)))
            gc += nb
        runs.append(sw_runs)
        sw_col_start.append(int(gc))
    total_cols = int(gc)
    nplanes = int(pc)
    max_sw_cols = max(sw_col_start[i + 1] - sw_col_start[i]
                      for i in range(len(sw_groups)))
    layout = {
        "runs": runs,
        "spans": spans,
        "plane_base": plane_base,
        "sw_groups": sw_groups,
        "sw_col_start": sw_col_start,
        "total_cols": total_cols,
        "nplanes": nplanes,
        "max_sw_cols": int(max_sw_cols),
        "smax": int(smax),
        "gmax": int(gmax),
    }

    sw_of_w = np.zeros(nw, np.int64)
    for si, grp in enumerate(sw_groups):
        for w in grp:
            sw_of_w[w] = si
    span_b0 = np.zeros((nw, NRANGE), np.int64)
    pb_arr = np.zeros((nw, NRANGE), np.int64)
    for w in range(nw):
        for r in range(NRANGE):
            span_b0[w, r] = spans[w][r][0]
            pb_arr[w, r] = plane_base[w][r]

    per_core = []
    for c in range(ncores):
        off, lcl, w, rix, key = cores[c]
        cnt = cnts[c]
        starts = np.zeros(nw * NRANGE, np.int64)
        starts[1:] = np.cumsum(cnt.ravel())[:-1]
        j = np.arange(len(lcl)) - starts[key]
        slot_in_run = seg_start[w, rix] + j
        rcol = run_col[sw_of_w[w], rix]
        col = rcol + slot_in_run // P
        pp = slot_in_run % P
        i16col = col * 8 + (slot_in_run % P) // 16
        i16row = slot_in_run % 16
        idx16 = np.zeros((16, total_cols * 8), np.int16)
        idx16[i16row, i16col] = off.astype(np.int16)
        idx16 = np.tile(idx16, (8, 1))
        # one-hot planes: plane for (w, rix) span, offset col - span_b0
        plane = pb_arr[w, rix] + (col - span_b0[w, rix])
        s_planes = np.zeros((P, nplanes, P), ml_dtypes.float8_e4m3)
        s_planes[pp, plane, lcl % P] = 1.0
        deg = np.bincount(lcl, minlength=nlocp).astype(np.float32)
        invd = (1.0 / np.maximum(deg, 1.0)).reshape(nw, P).T.copy()
        x_pad = np.zeros((nlocp, x.shape[1]), np.float32)
        x_pad[:nloc] = x[c * nloc:(c + 1) * nloc]
        per_core.append({
            "xT": np.ascontiguousarray(x_pad.T).astype(bf16),
            "idx16": idx16,
            "splanes": s_planes,
            "invd": invd.astype(np.float32),
        })
    return per_core, layout


def _run_pjrt(nc, in_maps, n_cores, bench_iters=0):
    """Execute the Bass program on the NeuronCores via PJRT/axon.

    Mirrors concourse.bass2jax.run_bass_via_pjrt, with an optional timing
    loop: inputs are pre-placed on device so repeated calls measure
    execute time (plus dispatch overhead) rather than host transfers.
    Returns (per_core_results, best_ns or None).
    """
    import time
    import jax
    import concourse.mybir as mybir
    from concourse.bass2jax import (_bass_exec_p, install_neuronx_cc_hook,
                                    partition_id_tensor)
    from jax.experimental.shard_map import shard_map
    from jax.sharding import Mesh, NamedSharding, PartitionSpec

    install_neuronx_cc_hook()

    partition_name = (nc.partition_id_tensor.name
                      if nc.partition_id_tensor else None)
    in_names, out_names, out_avals, zero_outs = [], [], [], []
    for alloc in nc.m.functions[0].allocations:
        if not isinstance(alloc, mybir.MemoryLocationSet):
            continue
        name = alloc.memorylocations[0].name
        if alloc.kind == "ExternalInput":
            if name != partition_name:
                in_names.append(name)
        elif alloc.kind == "ExternalOutput":
            shape = tuple(alloc.tensor_shape)
            dtype = mybir.dt.np(alloc.dtype)
            out_names.append(name)
            out_avals.append(jax.core.ShapedArray(shape, dtype))
            zero_outs.append(np.zeros(shape, dtype))
    n_params = len(in_names)
    n_outs = len(out_avals)
    in_names.extend(out_names)
    if partition_name is not None:
        in_names.append(partition_name)

    def _body(*args):
        operands = list(args)
        if partition_name is not None:
            operands.append(partition_id_tensor())
        return tuple(_bass_exec_p.bind(
            *operands,
            out_avals=tuple(out_avals),
            in_names=tuple(in_names),
            out_names=tuple(out_names),
            lowering_input_output_aliases=(),
            sim_require_finite=True,
            sim_require_nnan=True,
            nc=nc,
        ))

    devices = jax.devices()[:n_cores]
    assert len(devices) >= n_cores, devices
    mesh = Mesh(np.asarray(devices), ("core",))
    in_specs = (PartitionSpec("core"),) * (n_params + n_outs)
    out_specs = (PartitionSpec("core"),) * n_outs
    sharded = jax.jit(
        shard_map(_body, mesh=mesh, in_specs=in_specs, out_specs=out_specs,
                  check_rep=False),
        keep_unused=True)

    per_core = [[np.asarray(m[name]) for name in in_names[:n_params]]
                for m in in_maps]
    concat_in = [np.concatenate([per_core[c][i] for c in range(n_cores)],
                                axis=0) for i in range(n_params)]
    concat_zeros = [np.zeros((n_cores * z.shape[0], *z.shape[1:]), z.dtype)
                    for z in zero_outs]

    sharding = NamedSharding(mesh, PartitionSpec("core"))
    dev_in = [jax.device_put(a, sharding) for a in concat_in]
    dev_zeros = [jax.device_put(z, sharding) for z in concat_zeros]

    out_arrs = sharded(*dev_in, *dev_zeros)
    out_arrs = [np.asarray(o) for o in out_arrs]

    best_ns = None
    if bench_iters:
        # Device-time measurement through the axon tunnel: issue a chain
        # of async dispatches where each call's (unused) zero-output
        # operands come from the previous call's outputs. The device
        # serializes the NEFF executions while dispatch RPCs pipeline,
        # so (t_deep - t_shallow) / (n_deep - n_shallow) isolates
        # per-exec device time and cancels the round-trip.
        def chain(n):
            outs = tuple(dev_zeros)
            t0 = time.perf_counter()
            for _ in range(n):
                outs = sharded(*dev_in, *outs)
            for r in outs:
                r.block_until_ready()
            return time.perf_counter() - t0

        n1, n2 = 12, 12 + max(24, 2 * bench_iters)
        chain(4)  # warm
        diffs = []
        for _ in range(3):
            t1 = chain(n1)
            t2 = chain(n2)
            diffs.append((t2 - t1) / (n2 - n1) * 1e9)
        pos = sorted(d for d in diffs if d > 0)
        if pos:
            best_ns = pos[len(pos) // 2]
        else:
            best_ns = chain(n2) / n2 * 1e9

    results = [
        {name: out_arrs[i].reshape(n_cores, *out_avals[i].shape)[c]
         for i, name in enumerate(out_names)}
        for c in range(n_cores)
    ]
    return results, best_ns


def _make_shared(Wl0, Wr0, b0, Wl1, Wr1, b1, Wl2, Wr2, b2):
    bf16 = ml_dtypes.bfloat16
    Ws = [(np.asarray(Wl0, np.float32), np.asarray(Wr0, np.float32),
           np.asarray(b0, np.float32)),
          (np.asarray(Wl1, np.float32), np.asarray(Wr1, np.float32),
           np.asarray(b1, np.float32)),
          (np.asarray(Wl2, np.float32), np.asarray(Wr2, np.float32),
           np.asarray(b2, np.float32))]
    shared = {}
    for l, (Wl, Wr, b) in enumerate(Ws):
        shared[f"wcat{l}"] = np.ascontiguousarray(
            np.concatenate([Wl, Wr], axis=1).astype(bf16))
        shared[f"bbc{l}"] = np.ascontiguousarray(
            np.tile(b[None, :], (P, 1)).astype(np.float32))
    return shared


def kernel(x, edge_index, Wl0, Wr0, b0, Wl1, Wr1, b1, Wl2, Wr2, b2):
    global LAST_EXEC_TIME_NS, LAST_RESULTS

    x = np.ascontiguousarray(np.asarray(x, np.float32))
    ei = np.asarray(edge_index)
    src = ei[0].astype(np.int64)
    dst = ei[1].astype(np.int64)

    per_core, layout = _preprocess(x, src, dst)
    shared = _make_shared(Wl0, Wr0, b0, Wl1, Wr1, b1, Wl2, Wr2, b2)
    in_maps = [{**pc, **shared} for pc in per_core]

    nc = _build_program(layout)
    bench_iters = int(os.environ.get("GSAGE_BENCH_ITERS", "8"))
    results, best_ns = _run_pjrt(nc, in_maps, NCORES,
                                 bench_iters=bench_iters)
    LAST_EXEC_TIME_NS = best_ns
    LAST_RESULTS = results

    out = np.empty((N_NODES, OUT_CH), np.float32)
    for c in range(NCORES):
        out[c * NLOC:(c + 1) * NLOC] = results[c]["h_out"][:NLOC]
    return out
